# revision 2
# baseline (speedup 1.0000x reference)
"""Trainium2 Bass kernel for nn_AttentionBlock (B=4, S=2048, D=1024).

Sharding: 8 cores = 4 batches x 2 query-halves. Each core owns 1024
queries of one batch and produces y[own queries, 1024] directly in
row-major [q, f] layout. All matmuls in bf16 (fp32 PSUM accumulate),
rel err ~2e-3.

Host-side weight folds (x-independent):
    W2 = Wk^T @ Wq  ->  scoresT = X W2 Xq^T   ([key, query] layout)
    W3 = Wp @ Wv    ->  y = attn (X W3^T) = attn VP

Device phases (PE ~99% dense, ~169us of matmul):
  1:  G[g] = sum_d W2[g,d] @ Xq^T[d]                      (128 MMs)
  2a: VP for OWN keys only + pairwise AllGather (dedup):
  Each core computes VP = X @ W3^T only for its OWN 1024 keys (the global
  half it also owns as queries), bounces it to DRAM, and a pairwise
  AllGather ([[0,1],[2,3],[4,5],[6,7]]) assembles the full [2048, 1024]
  VP in GLOBAL key order on both cores — overlapped with the scores
  matmuls. This removes 128 duplicated 512-col matmuls (~28us PE).

SPMD rank-symmetry: the graph is identical on all cores, so "own keys"
enters as per-core input data (xot) and the gathered VP is in global key
order, matching the global-order expT tiles. Phase-4 reads all 16 VP
tiles from the gathered DRAM buffer.

Other changes vs v2: phase-4 passes [2,2,2,1,1] (smaller tail), epilogue
normalize on ScalarE (activation Copy with per-partition scale AP) with
only the bias add on DVE, first-tile DMA splits for an earlier first
matmul.
"""

import numpy as np
from contextlib import ExitStack

D = 1024
S = 2048
SQ = 1024  # queries per core
P = 128
ND = D // P   # 8
NS = S // P   # 16
SCALE = float(1.0 / np.sqrt(np.float32(D)).astype(np.float32))

_CACHED = {}


def _build_nc():
    import concourse.tile as tile
    from concourse import bacc, mybir

    BF = mybir.dt.bfloat16
    FP = mybir.dt.float32
    Exp = mybir.ActivationFunctionType.Exp
    Copy = mybir.ActivationFunctionType.Copy
    ADD = mybir.AluOpType.add

    nc = bacc.Bacc("TRN2", target_bir_lowering=False, num_devices=8)
    # xtr: host-rearranged X so each key tile sk is one contiguous row
    # block: xtr[sk*128 + p, d*128 + j] = X[sk*128 + j, d*128 + p].
    # One DMA push per key tile (16 total) instead of 128.
    xtr_d = nc.declare_dram_parameter("xtr", [S, D], BF, isOutput=False)
    xqt_d = nc.declare_dram_parameter("xqt", [D, SQ], BF, isOutput=False)
    w2t_d = nc.declare_dram_parameter("w2t", [D, D], BF, isOutput=False)
    w3t_d = nc.declare_dram_parameter("w3t", [D, D], BF, isOutput=False)
    biasb_d = nc.declare_dram_parameter("biasb", [P, D], FP, isOutput=False)
    onesc_d = nc.declare_dram_parameter("onesc", [P, 1], FP, isOutput=False)
    y_d = nc.declare_dram_parameter("y", [SQ, D], FP, isOutput=True)

    with tile.TileContext(nc) as tc:
        with ExitStack() as ctx:
            pool = ctx.enter_context(tc.tile_pool(name="main", bufs=1))
            psum = ctx.enter_context(tc.tile_pool(name="psum", bufs=1, space="PSUM"))
            dram = ctx.enter_context(tc.tile_pool(name="dram", bufs=1, space="DRAM"))

            def ptile(shape, name, tag, bufs=1, dt=BF):
                return pool.tile(shape, dt, name=name, tag=tag, bufs=bufs)

            def qbank(i, name, shape=(P, 1024)):
                return psum.tile(list(shape), FP, name=name, tag=f"q{i}", bufs=1)

            # ---- resident inputs ----
            xq = []
            w2t = []
            for d in range(ND):
                t = ptile([P, SQ], f"xq{d}", f"xq{d}")
                nc.sync.dma_start(t[:], xqt_d[d * P:(d + 1) * P, :])
                xq.append(t)
                t = ptile([P, D], f"w2t{d}", f"w2t{d}")
                nc.sync.dma_start(t[:], w2t_d[d * P:(d + 1) * P, :])
                w2t.append(t)
            w3t = []
            for d in range(ND):
                t = ptile([P, D], f"w3t{d}", f"w3t{d}")
                nc.sync.dma_start(t[:], w3t_d[d * P:(d + 1) * P, :])
                w3t.append(t)
            onesc_sb = ptile([P, 1], "onesc", "onesc", dt=FP)
            nc.sync.dma_start(onesc_sb[:], onesc_d[:, :])
            biasb_sb = ptile([P, D], "biasb", "biasb", dt=FP)
            nc.sync.dma_start(biasb_sb[:], biasb_d[:, :])
            # preload the full X^T key stream: all input DMA lands by ~31us,
            # well before the collective data phase hogs the DMA engines,
            # so the scores units never touch DMA mid-collective.
            xs_t = []
            for sk in range(NS):
                t = ptile([P, D], f"xs_{sk}", f"xs_{sk}")
                nc.sync.dma_start(t[:], xtr_d[sk * P:(sk + 1) * P, :])
                xs_t.append(t)

            # DRAM bounce buffers for the VP exchange
            own_vp_dram = dram.tile([SQ, D], BF)
            gath_vp_dram = dram.tile([S, D], BF)

            # ---- phase 1: G[g][128, SQ] = sum_d W2[g,d] @ Xq^T[d] ----
            g_sb = []
            for g in range(ND):
                g_sb.append(ptile([P, SQ], f"g{g}", f"g{g}"))
            for g in range(ND):
                pg = qbank(g % 4, f"pg_{g}")
                for d in range(ND):
                    lt = w2t[d][:, g * P:(g + 1) * P]
                    nc.tensor.matmul(pg[:, 0:512], lt, xq[d][:, 0:512],
                                     start=(d == 0), stop=(d == ND - 1))
                    nc.tensor.matmul(pg[:, 512:1024], lt, xq[d][:, 512:1024],
                                     start=(d == 0), stop=(d == ND - 1))
                nc.vector.tensor_copy(g_sb[g][:], pg[:])

            # ---- phase 2a: VP[j][128, 1024] = X_own[j-blk] @ W3^T for own
            #      keys only; bounce each tile to DRAM; pair-AllGather.
            #      All collective-path DMAs + the trigger live on the GPSIMD
            #      queue so they are not pushed behind the sync queue's
            #      ~160 input pushes.
            vp_own = []
            for j in range(ND):
                vp_own.append(ptile([P, D], f"vpo{j}", f"vpo{j % 4}", bufs=2))
            for j in range(ND):
                pvp = qbank(2 + j % 2, f"pv_{j}")
                for d in range(ND):
                    # own keys == own queries (same global half), so the
                    # stationary tile is a slice of the resident xq tiles
                    lt = xq[d][:, j * P:(j + 1) * P]
                    nc.tensor.matmul(pvp[:, 0:512], lt, w3t[d][:, 0:512],
                                     start=(d == 0), stop=(d == ND - 1))
                    nc.tensor.matmul(pvp[:, 512:1024], lt,
                                     w3t[d][:, 512:1024],
                                     start=(d == 0), stop=(d == ND - 1))
                nc.vector.tensor_copy(vp_own[j][:], pvp[:])
                nc.gpsimd.dma_start(own_vp_dram[j * P:(j + 1) * P, :],
                                    vp_own[j][:])
            nc.gpsimd.collective_compute(
                "AllGather",
                mybir.AluOpType.bypass,
                replica_groups=[[0, 1], [2, 3], [4, 5], [6, 7]],
                ins=[own_vp_dram[:].opt()],
                outs=[gath_vp_dram[:].opt()],
            )
            # gathered VP tiles (global key order) for phase 4, also on the
            # gpsimd queue (blocked there until the collective completes)
            vp = []
            for sk in range(NS):
                t = ptile([P, D], f"vp{sk}", f"vp{sk}")
                nc.gpsimd.dma_start(t[:], gath_vp_dram[sk * P:(sk + 1) * P, :])
                vp.append(t)

            # ---- phase 2b: scoresT[sk] = X^T[sk-blk].T @ G; exp; acc ----
            expT = []
            for sk in range(NS):
                expT.append(ptile([P, SQ], f"expT{sk}", f"expT{sk}"))
            acc_sb = ptile([P, SQ], "acc_sb", "acc_sb", dt=FP)
            for sk in range(NS):
                psc = qbank(sk % 2, f"psc_{sk}")
                for d in range(ND):
                    xs = xs_t[sk][:, d * P:(d + 1) * P]
                    nc.tensor.matmul(psc[:, 0:512], xs, g_sb[d][:, 0:512],
                                     start=(d == 0), stop=(d == ND - 1))
                    nc.tensor.matmul(psc[:, 512:1024], xs,
                                     g_sb[d][:, 512:1024],
                                     start=(d == 0), stop=(d == ND - 1))
                nc.scalar.activation(expT[sk][:], psc[:], Exp, scale=SCALE)
                if sk == 0:
                    nc.vector.tensor_copy(acc_sb[:], expT[0][:])
                else:
                    nc.vector.tensor_tensor(acc_sb[:], acc_sb[:], expT[sk][:], ADD)

            # ---- rowsum -> per-q-chunk reciprocal columns [128, 8] ----
            pr = qbank(0, "pr", shape=(P, 8))
            for c in range(ND):
                nc.tensor.matmul(pr[:, c:c + 1], acc_sb[:, c * P:(c + 1) * P],
                                 onesc_sb[:], start=True, stop=True)
            recip_sb = ptile([P, 8], "recip", "recip", dt=FP)
            nc.vector.reciprocal(recip_sb[:], pr[:, 0:8])

            # ---- phase 4: y[qc][128, 1024] = sum_sk expT[sk][:,qc]^T @ VP[sk]
            PASSES = [(0, 1), (2, 3), (4, 5), (6,), (7,)]
            for chunks in PASSES:
                pts = [qbank(c % 4, f"py_{c}") for c in chunks]
                for sk in range(NS):
                    for i, c in enumerate(chunks):
                        lt = expT[sk][:, c * P:(c + 1) * P]
                        nc.tensor.matmul(pts[i][:, 0:512], lt,
                                         vp[sk][:, 0:512],
                                         start=(sk == 0), stop=(sk == NS - 1))
                        nc.tensor.matmul(pts[i][:, 512:1024], lt,
                                         vp[sk][:, 512:1024],
                                         start=(sk == 0), stop=(sk == NS - 1))
                for i, c in enumerate(chunks):
                    ysb = ptile([P, D], f"ysb_{c}", f"ysb{c % 2}", dt=FP)
                    for h in range(2):
                        sl = slice(h * 512, (h + 1) * 512)
                        nc.scalar.activation(ysb[:, sl], pts[i][:, sl], Copy,
                                             scale=recip_sb[:, c:c + 1])
                        nc.vector.tensor_tensor(ysb[:, sl], ysb[:, sl],
                                                biasb_sb[:, sl], ADD)
                        nc.sync.dma_start(y_d[c * P:(c + 1) * P, sl], ysb[:, sl])

    nc.compile()
    return nc


def _get_nc():
    if "nc" not in _CACHED:
        _CACHED["nc"] = _build_nc()
    return _CACHED["nc"]


def make_in_maps(x, w_qkv, w_proj, b_proj):
    import ml_dtypes
    BF = ml_dtypes.bfloat16
    wq = w_qkv[0:D]
    wk = w_qkv[D:2 * D]
    wv = w_qkv[2 * D:3 * D]
    w2 = wk.T @ wq                   # scoresT = X W2 Xq^T
    w3 = w_proj @ wv                 # y = attn (X W3^T)
    w2T = np.ascontiguousarray(w2.T).astype(BF)
    w3T = np.ascontiguousarray(w3.T).astype(BF)
    biasb = np.ascontiguousarray(
        np.broadcast_to(b_proj[None, :], (P, D))).astype(np.float32)
    onesc = np.ones((P, 1), dtype=np.float32)
    in_maps = []
    for c in range(8):
        b, h = c // 2, c % 2
        # xtr[sk*128+p, d*128+j] = x[b][sk*128+j, d*128+p]
        xtr = np.ascontiguousarray(
            x[b].reshape(NS, P, ND, P).transpose(0, 3, 2, 1)
            .reshape(S, D)).astype(BF)
        xqt = np.ascontiguousarray(x[b, h * SQ:(h + 1) * SQ].T).astype(BF)
        in_maps.append({
            "xtr": xtr, "xqt": xqt, "w2t": w2T, "w3t": w3T,
            "biasb": biasb, "onesc": onesc,
        })
    return in_maps


def gather_out(results):
    out = np.empty((4, S, D), dtype=np.float32)
    for c in range(8):
        b, h = c // 2, c % 2
        out[b, h * SQ:(h + 1) * SQ] = results[c]["y"]
    return out


def kernel(x, w_qkv, w_proj, b_proj):
    from concourse import bass_utils
    nc = _get_nc()
    in_maps = make_in_maps(np.asarray(x, dtype=np.float32),
                           np.asarray(w_qkv, dtype=np.float32),
                           np.asarray(w_proj, dtype=np.float32),
                           np.asarray(b_proj, dtype=np.float32))
    res = bass_utils.run_bass_kernel_spmd(nc, in_maps, list(range(8))).results
    return gather_out(res)


# revision 3
# speedup vs baseline: 1.0034x; 1.0034x over previous
"""Trainium2 Bass kernel for nn_AttentionBlock (B=4, S=2048, D=1024).

Sharding: 8 cores = 4 batches x 2 query-halves. Each core owns 1024
queries of one batch and produces y[own queries, 1024] directly in
row-major [q, f] layout. All matmuls in bf16 (fp32 PSUM accumulate),
rel err ~2e-3.

Host-side weight folds (x-independent):
    W2 = Wk^T @ Wq  ->  scoresT = X W2 Xq^T   ([key, query] layout)
    W3 = Wp @ Wv    ->  y = attn (X W3^T) = attn VP

Device phases (PE ~99% dense, ~169us of matmul stream):
  1:  G[g] = sum_d W2[g,d] @ Xq^T[d]                          (128 MMs)
  2a: VP = X @ W3^T for the core's OWN 1024 keys only          (128 MMs)
      -> DRAM bounce -> pairwise AllGather [[0,1],[2,3],[4,5],[6,7]]
      assembles the full [2048, 1024] VP on both cores, overlapped with
      phase 2b. Removes the VP duplication across the batch pair
      (~28us of PE). SPMD rank-symmetry: own keys == own queries (same
      global half), and the gathered buffer is in GLOBAL key order for
      both pair members, so the shared graph stays rank-agnostic.
  2b: scoresT[sk] = X^T[sk].T @ G; exp on ScalarE -> bf16 expT;
      fp32 acc_sb += expT on DVE                               (256 MMs)
  rowsum: 8 tiny N=1 matmuls (stationary acc chunk, moving ones col)
      -> [128q, 1] per query chunk; one DVE reciprocal [128, 8]
  4:  y[qc] = sum_sk expT[sk][:, qc].T @ VP[sk]                (256 MMs)
      epilogue per chunk: ScalarE Copy with per-partition reciprocal
      scale, DVE bias add (pre-broadcast bias tile), DMA out.

Scheduling notes (hard-won):
  - All collective-path DMAs + the trigger live on the GPSIMD queue;
    the Sync sequencer pushes dma_starts strictly in order at ~0.6us
    each, so bulk pushes ahead of the trigger delay it by tens of us.
  - The full X^T key stream is preloaded (one push per key tile via the
    host-rearranged xtr layout); the AllGather data phase occupies all
    16 DMA engines for ~25us, and any compute that needs DMA during it
    starves.
"""

import numpy as np
from contextlib import ExitStack

D = 1024
S = 2048
SQ = 1024  # queries per core
P = 128
ND = D // P   # 8
NS = S // P   # 16
SCALE = float(1.0 / np.sqrt(np.float32(D)).astype(np.float32))

_CACHED = {}


def _build_nc():
    import concourse.tile as tile
    from concourse import bacc, mybir

    BF = mybir.dt.bfloat16
    FP = mybir.dt.float32
    Exp = mybir.ActivationFunctionType.Exp
    Copy = mybir.ActivationFunctionType.Copy
    ADD = mybir.AluOpType.add

    nc = bacc.Bacc("TRN2", target_bir_lowering=False, num_devices=8)
    # xtr: host-rearranged X so each key tile sk is one contiguous row
    # block: xtr[sk*128 + p, d*128 + j] = X[sk*128 + j, d*128 + p].
    # One DMA push per key tile (16 total) instead of 128.
    xtr_d = nc.declare_dram_parameter("xtr", [S, D], BF, isOutput=False)
    xqt_d = nc.declare_dram_parameter("xqt", [D, SQ], BF, isOutput=False)
    w2t_d = nc.declare_dram_parameter("w2t", [D, D], BF, isOutput=False)
    w3t_d = nc.declare_dram_parameter("w3t", [D, D], BF, isOutput=False)
    biasb_d = nc.declare_dram_parameter("biasb", [P, D], FP, isOutput=False)
    onesc_d = nc.declare_dram_parameter("onesc", [P, 1], FP, isOutput=False)
    y_d = nc.declare_dram_parameter("y", [SQ, D], FP, isOutput=True)

    with tile.TileContext(nc) as tc:
        with ExitStack() as ctx:
            pool = ctx.enter_context(tc.tile_pool(name="main", bufs=1))
            psum = ctx.enter_context(tc.tile_pool(name="psum", bufs=1, space="PSUM"))
            dram = ctx.enter_context(tc.tile_pool(name="dram", bufs=1, space="DRAM"))

            def ptile(shape, name, tag, bufs=1, dt=BF):
                return pool.tile(shape, dt, name=name, tag=tag, bufs=bufs)

            def qbank(i, name, shape=(P, 1024)):
                return psum.tile(list(shape), FP, name=name, tag=f"q{i}", bufs=1)

            # ---- resident inputs ----
            xq = []
            w2t = []
            for d in range(ND):
                t = ptile([P, SQ], f"xq{d}", f"xq{d}")
                nc.sync.dma_start(t[:], xqt_d[d * P:(d + 1) * P, :])
                xq.append(t)
                t = ptile([P, D], f"w2t{d}", f"w2t{d}")
                nc.sync.dma_start(t[:], w2t_d[d * P:(d + 1) * P, :])
                w2t.append(t)
            w3t = []
            for d in range(ND):
                t = ptile([P, D], f"w3t{d}", f"w3t{d}")
                nc.sync.dma_start(t[:], w3t_d[d * P:(d + 1) * P, :])
                w3t.append(t)
            onesc_sb = ptile([P, 1], "onesc", "onesc", dt=FP)
            nc.sync.dma_start(onesc_sb[:], onesc_d[:, :])
            biasb_sb = ptile([P, D], "biasb", "biasb", dt=FP)
            nc.sync.dma_start(biasb_sb[:], biasb_d[:, :])
            # preload the full X^T key stream: all input DMA lands by ~31us,
            # well before the collective data phase hogs the DMA engines,
            # so the scores units never touch DMA mid-collective.
            xs_t = []
            for sk in range(NS):
                t = ptile([P, D], f"xs_{sk}", f"xs_{sk}")
                nc.sync.dma_start(t[:], xtr_d[sk * P:(sk + 1) * P, :])
                xs_t.append(t)

            # DRAM bounce buffers for the VP exchange
            own_vp_dram = dram.tile([SQ, D], BF)
            gath_vp_dram = dram.tile([S, D], BF)

            # ---- phase 1: G[g][128, SQ] = sum_d W2[g,d] @ Xq^T[d] ----
            g_sb = []
            for g in range(ND):
                g_sb.append(ptile([P, SQ], f"g{g}", f"g{g}"))
            for g in range(ND):
                pg = qbank(g % 4, f"pg_{g}")
                for d in range(ND):
                    lt = w2t[d][:, g * P:(g + 1) * P]
                    nc.tensor.matmul(pg[:, 0:512], lt, xq[d][:, 0:512],
                                     start=(d == 0), stop=(d == ND - 1))
                    nc.tensor.matmul(pg[:, 512:1024], lt, xq[d][:, 512:1024],
                                     start=(d == 0), stop=(d == ND - 1))
                nc.vector.tensor_copy(g_sb[g][:], pg[:])

            # ---- phase 2a: VP[j][128, 1024] = X_own[j-blk] @ W3^T for own
            #      keys only; bounce each tile to DRAM; pair-AllGather.
            #      All collective-path DMAs + the trigger live on the GPSIMD
            #      queue so they are not pushed behind the sync queue's
            #      ~160 input pushes.
            vp_own = []
            for j in range(ND):
                vp_own.append(ptile([P, D], f"vpo{j}", f"vpo{j % 4}", bufs=2))
            for j in range(ND):
                pvp = qbank(2 + j % 2, f"pv_{j}")
                for d in range(ND):
                    # own keys == own queries (same global half), so the
                    # stationary tile is a slice of the resident xq tiles
                    lt = xq[d][:, j * P:(j + 1) * P]
                    nc.tensor.matmul(pvp[:, 0:512], lt, w3t[d][:, 0:512],
                                     start=(d == 0), stop=(d == ND - 1))
                    nc.tensor.matmul(pvp[:, 512:1024], lt,
                                     w3t[d][:, 512:1024],
                                     start=(d == 0), stop=(d == ND - 1))
                nc.vector.tensor_copy(vp_own[j][:], pvp[:])
                nc.gpsimd.dma_start(own_vp_dram[j * P:(j + 1) * P, :],
                                    vp_own[j][:])
            nc.gpsimd.collective_compute(
                "AllGather",
                mybir.AluOpType.bypass,
                replica_groups=[[0, 1], [2, 3], [4, 5], [6, 7]],
                ins=[own_vp_dram[:].opt()],
                outs=[gath_vp_dram[:].opt()],
            )
            # gathered VP tiles (global key order) for phase 4, also on the
            # gpsimd queue (blocked there until the collective completes)
            vp = []
            for sk in range(NS):
                t = ptile([P, D], f"vp{sk}", f"vp{sk}")
                nc.gpsimd.dma_start(t[:], gath_vp_dram[sk * P:(sk + 1) * P, :])
                vp.append(t)

            # ---- phase 2b: scoresT[sk] = X^T[sk-blk].T @ G; exp; acc ----
            expT = []
            for sk in range(NS):
                expT.append(ptile([P, SQ], f"expT{sk}", f"expT{sk}"))
            acc_sb = ptile([P, SQ], "acc_sb", "acc_sb", dt=FP)
            for sk in range(NS):
                psc = qbank(sk % 2, f"psc_{sk}")
                for d in range(ND):
                    xs = xs_t[sk][:, d * P:(d + 1) * P]
                    nc.tensor.matmul(psc[:, 0:512], xs, g_sb[d][:, 0:512],
                                     start=(d == 0), stop=(d == ND - 1))
                    nc.tensor.matmul(psc[:, 512:1024], xs,
                                     g_sb[d][:, 512:1024],
                                     start=(d == 0), stop=(d == ND - 1))
                nc.scalar.activation(expT[sk][:], psc[:], Exp, scale=SCALE)
                if sk == 0:
                    nc.vector.tensor_copy(acc_sb[:], expT[0][:])
                else:
                    nc.vector.tensor_tensor(acc_sb[:], acc_sb[:], expT[sk][:], ADD)

            # ---- rowsum -> per-q-chunk reciprocal columns [128, 8] ----
            pr = qbank(0, "pr", shape=(P, 8))
            for c in range(ND):
                nc.tensor.matmul(pr[:, c:c + 1], acc_sb[:, c * P:(c + 1) * P],
                                 onesc_sb[:], start=True, stop=True)
            recip_sb = ptile([P, 8], "recip", "recip", dt=FP)
            nc.vector.reciprocal(recip_sb[:], pr[:, 0:8])

            # ---- phase 4: y[qc][128, 1024] = sum_sk expT[sk][:,qc]^T @ VP[sk]
            PASSES = [(0, 1), (2, 3), (4, 5), (6,), (7,)]
            for chunks in PASSES:
                pts = [qbank(c % 4, f"py_{c}") for c in chunks]
                for sk in range(NS):
                    for i, c in enumerate(chunks):
                        lt = expT[sk][:, c * P:(c + 1) * P]
                        nc.tensor.matmul(pts[i][:, 0:512], lt,
                                         vp[sk][:, 0:512],
                                         start=(sk == 0), stop=(sk == NS - 1))
                        nc.tensor.matmul(pts[i][:, 512:1024], lt,
                                         vp[sk][:, 512:1024],
                                         start=(sk == 0), stop=(sk == NS - 1))
                for i, c in enumerate(chunks):
                    ysb = ptile([P, D], f"ysb_{c}", f"ysb{c % 2}", dt=FP)
                    for h in range(2):
                        sl = slice(h * 512, (h + 1) * 512)
                        nc.scalar.activation(ysb[:, sl], pts[i][:, sl], Copy,
                                             scale=recip_sb[:, c:c + 1])
                        nc.vector.tensor_tensor(ysb[:, sl], ysb[:, sl],
                                                biasb_sb[:, sl], ADD)
                        nc.sync.dma_start(y_d[c * P:(c + 1) * P, sl], ysb[:, sl])

    nc.compile()
    return nc


def _get_nc():
    if "nc" not in _CACHED:
        _CACHED["nc"] = _build_nc()
    return _CACHED["nc"]


def make_in_maps(x, w_qkv, w_proj, b_proj):
    import ml_dtypes
    BF = ml_dtypes.bfloat16
    wq = w_qkv[0:D]
    wk = w_qkv[D:2 * D]
    wv = w_qkv[2 * D:3 * D]
    w2 = wk.T @ wq                   # scoresT = X W2 Xq^T
    w3 = w_proj @ wv                 # y = attn (X W3^T)
    w2T = np.ascontiguousarray(w2.T).astype(BF)
    w3T = np.ascontiguousarray(w3.T).astype(BF)
    biasb = np.ascontiguousarray(
        np.broadcast_to(b_proj[None, :], (P, D))).astype(np.float32)
    onesc = np.ones((P, 1), dtype=np.float32)
    in_maps = []
    for c in range(8):
        b, h = c // 2, c % 2
        # xtr[sk*128+p, d*128+j] = x[b][sk*128+j, d*128+p]
        xtr = np.ascontiguousarray(
            x[b].reshape(NS, P, ND, P).transpose(0, 3, 2, 1)
            .reshape(S, D)).astype(BF)
        xqt = np.ascontiguousarray(x[b, h * SQ:(h + 1) * SQ].T).astype(BF)
        in_maps.append({
            "xtr": xtr, "xqt": xqt, "w2t": w2T, "w3t": w3T,
            "biasb": biasb, "onesc": onesc,
        })
    return in_maps


def gather_out(results):
    out = np.empty((4, S, D), dtype=np.float32)
    for c in range(8):
        b, h = c // 2, c % 2
        out[b, h * SQ:(h + 1) * SQ] = results[c]["y"]
    return out


def kernel(x, w_qkv, w_proj, b_proj):
    from concourse import bass_utils
    nc = _get_nc()
    in_maps = make_in_maps(np.asarray(x, dtype=np.float32),
                           np.asarray(w_qkv, dtype=np.float32),
                           np.asarray(w_proj, dtype=np.float32),
                           np.asarray(b_proj, dtype=np.float32))
    res = bass_utils.run_bass_kernel_spmd(nc, in_maps, list(range(8))).results
    return gather_out(res)
